# revision 31
# baseline (speedup 1.0000x reference)
"""Trainium2 Bass kernel for nn_MultiHeadAttn (sparse_attention).

Self-contained: accepts FULL unsharded inputs, returns FULL output.

Sharding: batch (2) x head-groups (4 heads each) -> 8 cores.
Each core computes, for its (batch b, heads 4g..4g+3):
  q/k projections -> qT/kT [d=64 per head, S] fp16 (pair-stacked tiles)
  v projection    -> v [S, 64 per head] fp16 (+ ones column for row-sums)
  scores^T[j,i] = k_j . q_i  (PE, K=64, per 128-row key block)
  + rel-pos bias + additive mask, injected via identity-matmul accumulation
  relu(x+spb)^2 (ACT relu + DVE square), unnormalized AV on PE,
  normalization folded into a post-AV per-query scale,
  output projection -> partial [S, 1024] f32, summed across head-groups on host.

Rel-pos bias B^T[j,i] = T[i, clamp(j-i,-16,16)+16], T = (q/8) @ rel_emb^T:
  - middle band (|i-j|<=~16 + clamped staircase corners) is materialized
    per (head, key-block) via a diagonal-affine DMA read from a DRAM
    "Text" expansion of T, then PE-transposed.
  - constant zones (T[i,32] left / T[i,0] right) are row-broadcasts,
    materialized once per head via partition-step-0 broadcast DMA.
"""
import sys

sys.path.insert(0, "/opt/trn_rl_repo")

import numpy as np
import concourse.bass as bass
import concourse.tile as tile
from concourse import bacc, mybir
from concourse.bass_utils import run_bass_kernel_spmd

fp16 = mybir.dt.float16
f32 = mybir.dt.float32
AF = mybir.ActivationFunctionType

S = 2048          # sequence length
DIN = 1024        # model dim
NH = 4            # heads per core
AD = 64           # head dim
KREL = 16
R = 288           # Text row stride (287 used + 1 pad)
MASKVAL = -30000.0
EPS = 1e-9

_NC_CACHE = {}


def build_nc():
    nc = bacc.Bacc("TRN2", target_bir_lowering=False, debug=False, num_devices=8)

    iqt_d = nc.dram_tensor("iqt", [DIN, S], fp16, kind="ExternalInput").ap()
    ikt_d = nc.dram_tensor("ikt", [DIN, S], fp16, kind="ExternalInput").ap()
    ivt_d = nc.dram_tensor("ivt", [DIN, S], fp16, kind="ExternalInput").ap()
    maskt_d = nc.dram_tensor("maskt", [S, S], fp16, kind="ExternalInput").ap()
    wqt_d = nc.dram_tensor("wqt", [DIN, NH * AD], fp16, kind="ExternalInput").ap()
    wkt_d = nc.dram_tensor("wkt", [DIN, NH * AD], fp16, kind="ExternalInput").ap()
    wvt_d = nc.dram_tensor("wvt", [DIN, NH * AD], fp16, kind="ExternalInput").ap()
    wot_d = nc.dram_tensor("wot", [NH * AD, DIN], fp16, kind="ExternalInput").ap()
    relt_d = nc.dram_tensor("relt", [128, 33], fp16, kind="ExternalInput").ap()
    ident_d = nc.dram_tensor("ident", [128, 128], fp16, kind="ExternalInput").ap()
    spb_d = nc.dram_tensor("spb", [1, 1], f32, kind="ExternalInput").ap()
    out_d = nc.dram_tensor("out", [S, DIN], f32, kind="ExternalOutput").ap()

    with tile.TileContext(nc, pool_alloc_mode="queue") as tc:
        _body(nc, tc, iqt_d, ikt_d, ivt_d, maskt_d, wqt_d, wkt_d, wvt_d, wot_d,
              relt_d, ident_d, spb_d, out_d)
    nc.compile()
    return nc


def _body(nc, tc, iqt_d, ikt_d, ivt_d, maskt_d, wqt_d, wkt_d, wvt_d, wot_d,
          relt_d, ident_d, spb_d, out_d):
    from contextlib import ExitStack
    with ExitStack() as ctx:
        dram = ctx.enter_context(tc.tile_pool(name="dram", bufs=1, space="DRAM"))
        singles = ctx.enter_context(tc.tile_pool(name="singles", bufs=1))

        text_d = [dram.tile([S, R], fp16, name=f"text{h}") for h in range(NH)]
        t01_d = [dram.tile([2, S], fp16, name=f"t01d{h}") for h in range(NH)]

        # ---- persistent SBUF ----
        maskt_all = singles.tile([128, 16, S], fp16)
        qt = [singles.tile([128, S], fp16, name=f"qt{p}") for p in range(2)]
        kt = [singles.tile([128, S], fp16, name=f"kt{p}") for p in range(2)]
        v_all = singles.tile([128, 16, NH * (AD + 1)], fp16)
        bnd_all = singles.tile([128, NH, 16, 160], fp16)
        oT_sb = [singles.tile([128, S], fp16, name=f"oTsb{p}") for p in range(2)]
        wot_sb = [singles.tile([128, DIN], fp16, name=f"wotsb{p}") for p in range(2)]
        ident = singles.tile([128, 128], fp16)
        relt_sb = singles.tile([128, 33], fp16)
        spb_bc = singles.tile([128, 1], f32)
        ones_t = singles.tile([128, 128], fp16)

        nc.sync.dma_start(ident, ident_d)
        nc.sync.dma_start(relt_sb, relt_d)
        nc.sync.dma_start(spb_bc[0:1, :], spb_d)
        nc.gpsimd.partition_broadcast(spb_bc, spb_bc[0:1, :], channels=128)
        nc.vector.memset(ones_t, 1.0)
        for p in range(2):
            nc.sync.dma_start(wot_sb[p], wot_d[p * 128:(p + 1) * 128, :])
        # ones columns of v_all (for row sums via augmented AV matmul)
        for h in range(NH):
            nc.vector.memset(v_all[:, :, h * (AD + 1) + AD:h * (AD + 1) + AD + 1], 1.0)

        def chunked_w(w_d):
            return bass.AP(tensor=w_d.tensor, offset=0,
                           ap=[[NH * AD, 128], [128 * NH * AD, 8], [1, NH * AD]])

        # ---- q/k projections ----
        with tc.tile_pool(name="pp", bufs=2, space="PSUM") as pp, \
             tc.tile_pool(name="stage", bufs=2) as stage, \
             tc.tile_pool(name="xstage", bufs=2) as xst:
            for (x_d, w_d, dst) in ((iqt_d, wqt_d, qt), (ikt_d, wkt_d, kt)):
                wstage = stage.tile([128, 8, NH * AD], fp16, tag="w")
                nc.sync.dma_start(wstage, chunked_w(w_d))
                ps = [pp.tile([128, S], f32, tag="pqk", name=f"ps{p}") for p in range(2)]
                for g in range(2):  # chunk-groups of 4, one staged load each
                    xt = xst.tile([128, 4, S], fp16, tag="x")
                    nc.sync.dma_start(
                        xt,
                        bass.AP(tensor=x_d.tensor, offset=g * 4 * 128 * S,
                                ap=[[S, 128], [128 * S, 4], [1, S]]))
                    for c4 in range(4):
                        ch = g * 4 + c4
                        for hp in range(2):
                            for n2 in range(4):
                                nc.tensor.matmul(
                                    ps[hp][:, n2 * 512:(n2 + 1) * 512],
                                    lhsT=wstage[:, ch, hp * 128:(hp + 1) * 128],
                                    rhs=xt[:, c4, n2 * 512:(n2 + 1) * 512],
                                    start=(ch == 0), stop=(ch == 7))
                for hp in range(2):
                    nc.scalar.activation(dst[hp], ps[hp], AF.Copy)

        # ---- v projection + T/Text/band, all concurrently scheduled ----
        # psum: pv 2x[128,256](2 banks) + T 2x(1) + tps 2x(1) + bps 2x(1) = 8
        with tc.tile_pool(name="pv", bufs=2, space="PSUM") as pv, \
             tc.tile_pool(name="vw", bufs=1) as vw, \
             tc.tile_pool(name="pt", bufs=2, space="PSUM") as pt, \
             tc.tile_pool(name="textp", bufs=3) as textp, \
             tc.tile_pool(name="tcolp", bufs=2) as tcolp, \
             tc.tile_pool(name="pb", bufs=2, space="PSUM") as pb, \
             tc.tile_pool(name="bst", bufs=3) as bst:
            wvstage = vw.tile([128, 8, NH * AD], fp16)
            nc.sync.dma_start(wvstage, chunked_w(wvt_d))
            xv_all = vw.tile([128, 8, S], fp16)
            nc.sync.dma_start(
                xv_all,
                bass.AP(tensor=ivt_d.tensor, offset=0,
                        ap=[[S, 128], [128 * S, 8], [1, S]]))
            for kb in range(16):
                v_ps = pv.tile([128, NH * AD], f32, tag="pv")
                for ch in range(8):
                    nc.tensor.matmul(
                        v_ps, lhsT=xv_all[:, ch, kb * 128:(kb + 1) * 128],
                        rhs=wvstage[:, ch, :], start=(ch == 0), stop=(ch == 7))
                nc.scalar.activation(
                    v_all[:, kb, :].rearrange("p (h c) -> p h c", c=AD + 1)[:, :, 0:AD],
                    v_ps.rearrange("p (h c) -> p h c", c=AD), AF.Copy)

            for h in range(NH):
                b = 64 * (h % 2)
                # T = q @ rel_emb^T; Text row expansion -> DRAM (4 chunks/DMA)
                tcols = tcolp.tile([128, 2, 16], f32, tag="tcols")
                for i4 in range(4):
                    text_t = textp.tile([128, 4, R], fp16, tag="text")
                    for c4 in range(4):
                        ic = i4 * 4 + c4
                        T_ps = pt.tile([128, 33], f32, tag="T")
                        nc.tensor.matmul(
                            T_ps, lhsT=qt[h // 2][b:b + 64, ic * 128:(ic + 1) * 128],
                            rhs=relt_sb[b:b + 64, :], start=True, stop=True)
                        tt = text_t[:, c4, :]
                        nc.scalar.activation(tt[:, 127:160], T_ps, AF.Copy)
                        nc.vector.tensor_copy(tcols[:, 0, ic:ic + 1], T_ps[:, 0:1])
                        nc.vector.tensor_copy(tcols[:, 1, ic:ic + 1], T_ps[:, 32:33])
                        nc.vector.tensor_scalar_mul(
                            tt[:, 0:127], ones_t[:, 0:127], tcols[:, 0, ic:ic + 1])
                        nc.vector.tensor_scalar_mul(
                            tt[:, 160:287], ones_t[:, 0:127], tcols[:, 1, ic:ic + 1])
                    nc.sync.dma_start(
                        bass.AP(tensor=text_d[h].tensor, offset=i4 * 4 * 128 * R,
                                ap=[[R, 128], [128 * R, 4], [1, R]]),
                        text_t)
                # transpose tcols -> [32, 128]: rows 0-15 = T0 segments,
                # rows 16-31 = T32 segments; ship rows to DRAM as [2, S]
                tcols16 = tcolp.tile([128, 32], fp16, tag="tcols16")
                nc.vector.tensor_copy(tcols16, tcols.rearrange("p a b -> p (a b)"))
                tps = pt.tile([32, 128], fp16, tag="tps")
                nc.tensor.transpose(tps, tcols16, ident)
                tsb = tcolp.tile([32, 128], fp16, tag="tsb")
                nc.vector.tensor_copy(tsb, tps)
                nc.sync.dma_start(t01_d[h], tsb)
                # band tiles: diagonal DMA reads of Text (batched) + PE transpose
                bba_all = bst.tile([128, 14, 128], fp16, tag="bba_all")
                nc.sync.dma_start(
                    bba_all,
                    bass.AP(tensor=text_d[h].tensor,
                            offset=112 * (R - 1) + 128 + 143,
                            ap=[[R - 1, 128], [128 * (R - 1) + 128, 14], [1, 128]]))
                bbb_all = bst.tile([32, 14, 128], fp16, tag="bbb_all")
                nc.sync.dma_start(
                    bbb_all,
                    bass.AP(tensor=text_d[h].tensor,
                            offset=(112 + 128) * (R - 1) + 128 + 143,
                            ap=[[R - 1, 32], [128 * (R - 1) + 128, 14], [1, 128]]))
                for kb in range(16):
                    j0 = kb * 128
                    i_start = max(j0 - 16, 0)
                    nrows = min(j0 + 144, S) - i_start
                    base_off = i_start * (R - 1) + j0 + 143
                    b_ps = pb.tile([128, 160], fp16, tag="bps")
                    if 1 <= kb <= 14:
                        nc.tensor.transpose(
                            b_ps[:, 0:128], bba_all[:, kb - 1, :], ident)
                        nc.tensor.transpose(
                            b_ps[:, 128:160], bbb_all[0:32, kb - 1, :],
                            ident[0:32, 0:32])
                    else:
                        bba = bst.tile([128, 128], fp16, tag="bba")
                        nc.sync.dma_start(
                            bba,
                            bass.AP(tensor=text_d[h].tensor, offset=base_off,
                                    ap=[[R - 1, 128], [1, 128]]))
                        nc.tensor.transpose(b_ps[:, 0:128], bba, ident)
                        n2 = nrows - 128
                        bbb = bst.tile([64, 128], fp16, tag="bbb")
                        nc.sync.dma_start(
                            bbb[0:n2, :],
                            bass.AP(tensor=text_d[h].tensor,
                                    offset=base_off + 128 * (R - 1),
                                    ap=[[R - 1, n2], [1, 128]]))
                        nc.tensor.transpose(
                            b_ps[:, 128:128 + n2], bbb[0:n2, :], ident[0:n2, 0:n2])
                    nc.scalar.activation(
                        bnd_all[:, h, kb, 0:nrows], b_ps[:, 0:nrows], AF.Copy)
            # whole mask in one chunked 3D DMA (emitted late so it doesn't
            # block the projection input loads on the DMA queues)
            nc.sync.dma_start(
                maskt_all,
                bass.AP(tensor=maskt_d.tensor, offset=0,
                        ap=[[S, 128], [128 * S, 16], [1, S]]))

        import os
        if os.environ.get("K_PHASE") == "setup":
            return
        # ---- main attention loop (head pairs row-packed on the PE) ----
        with tc.tile_pool(name="sp", bufs=2, space="PSUM") as sp, \
             tc.tile_pool(name="op", bufs=2, space="PSUM") as op, \
             tc.tile_pool(name="zp", bufs=2) as zp, \
             tc.tile_pool(name="mp", bufs=4) as mp, \
             tc.tile_pool(name="rp", bufs=3) as rp, \
             tc.tile_pool(name="tailp", bufs=2) as tailp:

            def emit_merge(eng, dst, h, kb, Q0, z_lo, z_hi):
                # dst [128,1024] fp16 = maskt + (zone | band) for cols [Q0,Q0+1024)
                j0 = kb * 128
                c1 = max(j0 - 16, 0)
                c2 = min(j0 + 144, S)
                Q1 = Q0 + 1024
                lo_a, lo_b = Q0, min(c1, Q1)
                if lo_a < lo_b:
                    eng.tensor_add(dst[:, lo_a - Q0:lo_b - Q0],
                                   maskt_all[:, kb, lo_a:lo_b], z_lo[:, lo_a:lo_b])
                bd_a, bd_b = max(c1, Q0), min(c2, Q1)
                if bd_a < bd_b:
                    eng.tensor_add(dst[:, bd_a - Q0:bd_b - Q0],
                                   maskt_all[:, kb, bd_a:bd_b],
                                   bnd_all[:, h, kb, bd_a - c1:bd_b - c1])
                hi_a, hi_b = max(c2, Q0), Q1
                if hi_a < hi_b:
                    eng.tensor_add(dst[:, hi_a - Q0:hi_b - Q0],
                                   maskt_all[:, kb, hi_a:hi_b], z_hi[:, hi_a:hi_b])

            for hp in range(2):
                # zone rows: strided column reads of Text, then broadcast.
                # left cols (i <= j-17) -> T[i,32] (Text col 160);
                # right cols (i >= j+17) -> T[i,0] (Text col 0)
                zs = []
                for e in range(2):
                    h = 2 * hp + e
                    z_lo = zp.tile([128, S], fp16, tag=f"zlo{e}", name=f"zlo{e}")
                    nc.sync.dma_start(z_lo[0:1, :], t01_d[h][1:2, :])
                    nc.gpsimd.partition_broadcast(z_lo, z_lo[0:1, :], channels=128)
                    z_hi = zp.tile([128, S], fp16, tag=f"zhi{e}", name=f"zhi{e}")
                    nc.sync.dma_start(z_hi[0:1, :], t01_d[h][0:1, :])
                    nc.gpsimd.partition_broadcast(z_hi, z_hi[0:1, :], channels=128)
                    zs.append((z_lo, z_hi))
                for qh in range(2):
                    Q0 = qh * 1024
                    Q1 = Q0 + 1024
                    oTs = [op.tile([AD + 1, 1024], f32, tag="ot", name=f"ot{e}")
                           for e in range(2)]
                    for kb in range(16):
                        j0 = kb * 128
                        s_tiles = []
                        for e in range(2):
                            h = 2 * hp + e
                            m_mg = mp.tile([128, 1024], fp16, tag="mg",
                                           name=f"mg{e}")
                            eng = nc.gpsimd if (kb + e) % 2 == 1 else nc.vector
                            emit_merge(eng, m_mg, h, kb, Q0, *zs[e])
                            s_ps = sp.tile([128, 1024], f32, tag="s", name=f"s{e}")
                            b = 64 * e
                            for q2 in range(2):
                                qs = Q0 + q2 * 512
                                nc.tensor.matmul(
                                    s_ps[:, q2 * 512:(q2 + 1) * 512],
                                    lhsT=kt[hp][b:b + 64, j0:j0 + 128],
                                    rhs=qt[hp][b:b + 64, qs:qs + 512],
                                    start=True, stop=False, skip_group_check=True)
                                nc.tensor.matmul(
                                    s_ps[:, q2 * 512:(q2 + 1) * 512],
                                    lhsT=ident,
                                    rhs=m_mg[:, q2 * 512:(q2 + 1) * 512],
                                    start=False, stop=True, skip_group_check=True)
                            s_tiles.append(s_ps)
                        for e in range(2):
                            h = 2 * hp + e
                            g0 = h * (AD + 1)
                            r_t = rp.tile([128, 1024], fp16, tag="r", name=f"r{e}")
                            nc.scalar.activation(r_t, s_tiles[e], AF.Relu,
                                                 bias=spb_bc[:, 0:1])
                            t_t = rp.tile([128, 1024], fp16, tag="t", name=f"t{e}")
                            nc.vector.tensor_mul(t_t, r_t, r_t)
                            for q2 in range(2):
                                nc.tensor.matmul(
                                    oTs[e][:, q2 * 512:(q2 + 1) * 512],
                                    lhsT=v_all[:, kb, g0:g0 + AD + 1],
                                    rhs=t_t[:, q2 * 512:(q2 + 1) * 512],
                                    start=(kb == 0), stop=(kb == 15),
                                    skip_group_check=True)
                    # normalization: rc = 1/(sum + eps), broadcast, scale
                    for e in range(2):
                        oT_ps = oTs[e]
                        rcb = tailp.tile([64, 1024], f32, tag="rcb", name="rcb")
                        nc.vector.tensor_scalar_add(
                            rcb[0:1, :], oT_ps[AD:AD + 1, :], EPS)
                        nc.vector.reciprocal(rcb[0:1, :], rcb[0:1, :])
                        nc.gpsimd.partition_broadcast(rcb, rcb[0:1, :], channels=64)
                        if e == 0:
                            nc.vector.tensor_mul(
                                oT_sb[hp][0:64, Q0:Q1], oT_ps[0:AD, :], rcb)
                        else:
                            ost = tailp.tile([64, 1024], fp16, tag="ost",
                                             name="ost")
                            nc.vector.tensor_mul(ost, oT_ps[0:AD, :], rcb)
                            nc.sync.dma_start(oT_sb[hp][64:128, Q0:Q1], ost)

        if os.environ.get("K_PHASE") == "nomain_out":
            pass
        # ---- output projection ----
        with tc.tile_pool(name="po", bufs=2, space="PSUM") as po, \
             tc.tile_pool(name="outp", bufs=3) as outp:
            for qb in range(16):
                o_ps = po.tile([128, DIN], f32, tag="o")
                for ci in range(2):
                    for n2 in range(2):
                        nc.tensor.matmul(
                            o_ps[:, n2 * 512:(n2 + 1) * 512],
                            lhsT=oT_sb[ci][:, qb * 128:(qb + 1) * 128],
                            rhs=wot_sb[ci][:, n2 * 512:(n2 + 1) * 512],
                            start=(ci == 0), stop=(ci == 1))
                outsb = outp.tile([128, DIN], f32, tag="outsb")
                nc.scalar.activation(outsb, o_ps, AF.Copy)
                nc.sync.dma_start(out_d[qb * 128:(qb + 1) * 128, :], outsb)


def _prep_core_inputs(c, iQ, iK, iV, mask, Wq, Wk, Wv, Wo, rel_emb, sp_bias):
    b, g = divmod(c, 4)
    hs = slice(g * NH * AD, (g + 1) * NH * AD)
    relt = np.ascontiguousarray(rel_emb.T).astype(np.float16)  # [64, 33]
    return {
        "iqt": np.ascontiguousarray(iQ[b].T).astype(np.float16),
        "ikt": np.ascontiguousarray(iK[b].T).astype(np.float16),
        "ivt": np.ascontiguousarray(iV[b].T).astype(np.float16),
        "maskt": np.where(mask[b].T, np.float16(MASKVAL), np.float16(0.0)),
        "wqt": np.ascontiguousarray((Wq[hs, :] / 8.0).T).astype(np.float16),
        "wkt": np.ascontiguousarray(Wk[hs, :].T).astype(np.float16),
        "wvt": np.ascontiguousarray(Wv[hs, :].T).astype(np.float16),
        "wot": np.ascontiguousarray(Wo[:, hs].T).astype(np.float16),
        "relt": np.concatenate([relt, relt], axis=0),  # [128, 33] (both halves)
        "ident": np.eye(128, dtype=np.float16),
        "spb": np.asarray(sp_bias, dtype=np.float32).reshape(1, 1),
    }


def kernel(iQ, iK, iV, mask, Wq, Wk, Wv, Wo, rel_emb, sp_bias, _trace=False,
           **_run_kwargs):
    iQ, iK, iV = (np.asarray(x, np.float32) for x in (iQ, iK, iV))
    mask = np.asarray(mask, bool)
    Wq, Wk, Wv, Wo = (np.asarray(x, np.float32) for x in (Wq, Wk, Wv, Wo))
    rel_emb = np.asarray(rel_emb, np.float32)
    sp_bias = np.asarray(sp_bias, np.float32)

    if "nc" not in _NC_CACHE:
        _NC_CACHE["nc"] = build_nc()
    nc = _NC_CACHE["nc"]

    in_maps = [
        _prep_core_inputs(c, iQ, iK, iV, mask, Wq, Wk, Wv, Wo, rel_emb, sp_bias)
        for c in range(8)
    ]
    res = run_bass_kernel_spmd(nc, in_maps, core_ids=list(range(8)),
                               trace=_trace, **_run_kwargs)
    out = np.zeros((2, S, DIN), np.float32)
    for c in range(8):
        bi = c // 4
        out[bi] += res.results[c]["out"]
    kernel.last_result = res
    return out


# revision 36
# speedup vs baseline: 14734.8412x; 14734.8412x over previous
"""Trainium2 Bass kernel for nn_MultiHeadAttn (sparse_attention).

Self-contained: accepts FULL unsharded inputs, returns FULL output.

Sharding: batch (2) x head-groups (4 heads each) -> 8 cores.
Each core computes, for its (batch b, heads 4g..4g+3):
  q/k projections -> qT/kT [d=64 per head, S] fp16 (pair-stacked tiles)
  v projection    -> v [S, 64 per head] fp16 (+ ones column for row-sums)
  scores^T[j,i] = k_j . q_i  (PE, K=64, per 128-row key block)
  + rel-pos bias + additive mask, injected via identity-matmul accumulation
  relu(x+spb)^2 (ACT relu + DVE square), unnormalized AV on PE,
  normalization folded into a post-AV per-query scale,
  output projection -> partial [S, 1024] f32, summed across head-groups on host.

Rel-pos bias B^T[j,i] = T[i, clamp(j-i,-16,16)+16], T = (q/8) @ rel_emb^T:
  - middle band (|i-j|<=~16 + clamped staircase corners) is materialized
    per (head, key-block) via a diagonal-affine DMA read from a DRAM
    "Text" expansion of T, then PE-transposed.
  - constant zones (T[i,32] left / T[i,0] right) are row-broadcasts,
    materialized once per head via partition-step-0 broadcast DMA.
"""
import sys

sys.path.insert(0, "/opt/trn_rl_repo")

import numpy as np
import concourse.bass as bass
import concourse.tile as tile
from concourse import bacc, mybir
from concourse.bass_utils import run_bass_kernel_spmd

fp16 = mybir.dt.float16
f32 = mybir.dt.float32
AF = mybir.ActivationFunctionType

S = 2048          # sequence length
DIN = 1024        # model dim
NH = 4            # heads per core
AD = 64           # head dim
KREL = 16
R = 288           # Text row stride (287 used + 1 pad)
MASKVAL = -30000.0
EPS = 1e-9

_NC_CACHE = {}


def build_nc():
    nc = bacc.Bacc("TRN2", target_bir_lowering=False, debug=False, num_devices=8)

    iqt_d = nc.dram_tensor("iqt", [DIN, S], fp16, kind="ExternalInput").ap()
    ikt_d = nc.dram_tensor("ikt", [DIN, S], fp16, kind="ExternalInput").ap()
    ivt_d = nc.dram_tensor("ivt", [DIN, S], fp16, kind="ExternalInput").ap()
    maskt_d = nc.dram_tensor("maskt", [S, S], fp16, kind="ExternalInput").ap()
    wqt_d = nc.dram_tensor("wqt", [DIN, NH * AD], fp16, kind="ExternalInput").ap()
    wkt_d = nc.dram_tensor("wkt", [DIN, NH * AD], fp16, kind="ExternalInput").ap()
    wvt_d = nc.dram_tensor("wvt", [DIN, NH * AD], fp16, kind="ExternalInput").ap()
    wot_d = nc.dram_tensor("wot", [NH * AD, DIN], fp16, kind="ExternalInput").ap()
    relt_d = nc.dram_tensor("relt", [128, 33], fp16, kind="ExternalInput").ap()
    ident_d = nc.dram_tensor("ident", [128, 128], fp16, kind="ExternalInput").ap()
    spb_d = nc.dram_tensor("spb", [1, 1], f32, kind="ExternalInput").ap()
    out_d = nc.dram_tensor("out", [S, DIN], f32, kind="ExternalOutput").ap()

    with tile.TileContext(nc, pool_alloc_mode="queue") as tc:
        _body(nc, tc, iqt_d, ikt_d, ivt_d, maskt_d, wqt_d, wkt_d, wvt_d, wot_d,
              relt_d, ident_d, spb_d, out_d)
    nc.compile()
    return nc


def _body(nc, tc, iqt_d, ikt_d, ivt_d, maskt_d, wqt_d, wkt_d, wvt_d, wot_d,
          relt_d, ident_d, spb_d, out_d):
    from contextlib import ExitStack
    with ExitStack() as ctx:
        dram = ctx.enter_context(tc.tile_pool(name="dram", bufs=1, space="DRAM"))
        singles = ctx.enter_context(tc.tile_pool(name="singles", bufs=1))

        text_d = [dram.tile([S, R], fp16, name=f"text{h}") for h in range(NH)]
        t01_d = [dram.tile([2, S], fp16, name=f"t01d{h}") for h in range(NH)]

        # ---- persistent SBUF ----
        maskt_all = singles.tile([128, 16, S], fp16)
        qt = [singles.tile([128, S], fp16, name=f"qt{p}") for p in range(2)]
        kt = [singles.tile([128, S], fp16, name=f"kt{p}") for p in range(2)]
        v_all = singles.tile([128, 16, NH * (AD + 1)], fp16)
        bnd_all = singles.tile([128, NH, 16, 160], fp16)
        oT_sb = [singles.tile([128, S], fp16, name=f"oTsb{p}") for p in range(2)]
        wot_sb = [singles.tile([128, DIN], fp16, name=f"wotsb{p}") for p in range(2)]
        ident = singles.tile([128, 128], fp16)
        relt_sb = singles.tile([128, 33], fp16)
        spb_bc = singles.tile([128, 1], f32)
        ones_t = singles.tile([128, 128], fp16)

        nc.sync.dma_start(ident, ident_d)
        nc.sync.dma_start(relt_sb, relt_d)
        nc.sync.dma_start(spb_bc[0:1, :], spb_d)
        nc.gpsimd.partition_broadcast(spb_bc, spb_bc[0:1, :], channels=128)
        nc.vector.memset(ones_t, 1.0)
        for p in range(2):
            nc.sync.dma_start(wot_sb[p], wot_d[p * 128:(p + 1) * 128, :])
        # ones columns of v_all (for row sums via augmented AV matmul)
        for h in range(NH):
            nc.vector.memset(v_all[:, :, h * (AD + 1) + AD:h * (AD + 1) + AD + 1], 1.0)

        def chunked_w(w_d):
            return bass.AP(tensor=w_d.tensor, offset=0,
                           ap=[[NH * AD, 128], [128 * NH * AD, 8], [1, NH * AD]])

        # ---- q/k projections ----
        with tc.tile_pool(name="pp", bufs=2, space="PSUM") as pp, \
             tc.tile_pool(name="stage", bufs=2) as stage, \
             tc.tile_pool(name="xstage", bufs=2) as xst:
            for (x_d, w_d, dst) in ((iqt_d, wqt_d, qt), (ikt_d, wkt_d, kt)):
                wstage = stage.tile([128, 8, NH * AD], fp16, tag="w")
                nc.sync.dma_start(wstage, chunked_w(w_d))
                ps = [pp.tile([128, S], f32, tag="pqk", name=f"ps{p}") for p in range(2)]
                for g in range(2):  # chunk-groups of 4, one staged load each
                    xt = xst.tile([128, 4, S], fp16, tag="x")
                    nc.sync.dma_start(
                        xt,
                        bass.AP(tensor=x_d.tensor, offset=g * 4 * 128 * S,
                                ap=[[S, 128], [128 * S, 4], [1, S]]))
                    for c4 in range(4):
                        ch = g * 4 + c4
                        for hp in range(2):
                            for n2 in range(4):
                                nc.tensor.matmul(
                                    ps[hp][:, n2 * 512:(n2 + 1) * 512],
                                    lhsT=wstage[:, ch, hp * 128:(hp + 1) * 128],
                                    rhs=xt[:, c4, n2 * 512:(n2 + 1) * 512],
                                    start=(ch == 0), stop=(ch == 7))
                for hp in range(2):
                    nc.scalar.activation(dst[hp], ps[hp], AF.Copy)

        # ---- v projection + T/Text/band, all concurrently scheduled ----
        # psum: pv 2x[128,256](2 banks) + T 2x(1) + tps 2x(1) + bps 2x(1) = 8
        with tc.tile_pool(name="pv", bufs=2, space="PSUM") as pv, \
             tc.tile_pool(name="vw", bufs=1) as vw, \
             tc.tile_pool(name="pt", bufs=2, space="PSUM") as pt, \
             tc.tile_pool(name="textp", bufs=3) as textp, \
             tc.tile_pool(name="tcolp", bufs=2) as tcolp, \
             tc.tile_pool(name="pb", bufs=2, space="PSUM") as pb, \
             tc.tile_pool(name="bst", bufs=3) as bst:
            wvstage = vw.tile([128, 8, NH * AD], fp16)
            nc.sync.dma_start(wvstage, chunked_w(wvt_d))
            xv_all = vw.tile([128, 8, S], fp16)
            nc.sync.dma_start(
                xv_all,
                bass.AP(tensor=ivt_d.tensor, offset=0,
                        ap=[[S, 128], [128 * S, 8], [1, S]]))
            for kb in range(16):
                v_ps = pv.tile([128, NH * AD], f32, tag="pv")
                for ch in range(8):
                    nc.tensor.matmul(
                        v_ps, lhsT=xv_all[:, ch, kb * 128:(kb + 1) * 128],
                        rhs=wvstage[:, ch, :], start=(ch == 0), stop=(ch == 7))
                nc.scalar.activation(
                    v_all[:, kb, :].rearrange("p (h c) -> p h c", c=AD + 1)[:, :, 0:AD],
                    v_ps.rearrange("p (h c) -> p h c", c=AD), AF.Copy)

            for h in range(NH):
                b = 64 * (h % 2)
                # T = q @ rel_emb^T; Text row expansion -> DRAM (4 chunks/DMA)
                tcols = tcolp.tile([128, 2, 16], f32, tag="tcols")
                for i4 in range(4):
                    text_t = textp.tile([128, 4, R], fp16, tag="text")
                    for c4 in range(4):
                        ic = i4 * 4 + c4
                        T_ps = pt.tile([128, 33], f32, tag="T")
                        nc.tensor.matmul(
                            T_ps, lhsT=qt[h // 2][b:b + 64, ic * 128:(ic + 1) * 128],
                            rhs=relt_sb[b:b + 64, :], start=True, stop=True)
                        tt = text_t[:, c4, :]
                        nc.scalar.activation(tt[:, 127:160], T_ps, AF.Copy)
                        nc.vector.tensor_copy(tcols[:, 0, ic:ic + 1], T_ps[:, 0:1])
                        nc.vector.tensor_copy(tcols[:, 1, ic:ic + 1], T_ps[:, 32:33])
                        nc.vector.tensor_scalar_mul(
                            tt[:, 0:127], ones_t[:, 0:127], tcols[:, 0, ic:ic + 1])
                        nc.vector.tensor_scalar_mul(
                            tt[:, 160:287], ones_t[:, 0:127], tcols[:, 1, ic:ic + 1])
                    nc.sync.dma_start(
                        bass.AP(tensor=text_d[h].tensor, offset=i4 * 4 * 128 * R,
                                ap=[[R, 128], [128 * R, 4], [1, R]]),
                        text_t)
                # transpose tcols -> [32, 128]: rows 0-15 = T0 segments,
                # rows 16-31 = T32 segments; ship rows to DRAM as [2, S]
                tcols16 = tcolp.tile([128, 32], fp16, tag="tcols16")
                nc.vector.tensor_copy(tcols16, tcols.rearrange("p a b -> p (a b)"))
                tps = pt.tile([32, 128], fp16, tag="tps")
                nc.tensor.transpose(tps, tcols16, ident)
                tsb = tcolp.tile([32, 128], fp16, tag="tsb")
                nc.vector.tensor_copy(tsb, tps)
                nc.sync.dma_start(t01_d[h], tsb)
                # band tiles: diagonal DMA reads of Text (batched) + PE transpose
                bba_all = bst.tile([128, 14, 128], fp16, tag="bba_all")
                nc.sync.dma_start(
                    bba_all,
                    bass.AP(tensor=text_d[h].tensor,
                            offset=112 * (R - 1) + 128 + 143,
                            ap=[[R - 1, 128], [128 * (R - 1) + 128, 14], [1, 128]]))
                bbb_all = bst.tile([32, 14, 128], fp16, tag="bbb_all")
                nc.sync.dma_start(
                    bbb_all,
                    bass.AP(tensor=text_d[h].tensor,
                            offset=(112 + 128) * (R - 1) + 128 + 143,
                            ap=[[R - 1, 32], [128 * (R - 1) + 128, 14], [1, 128]]))
                for kb in range(16):
                    j0 = kb * 128
                    i_start = max(j0 - 16, 0)
                    nrows = min(j0 + 144, S) - i_start
                    base_off = i_start * (R - 1) + j0 + 143
                    b_ps = pb.tile([128, 160], fp16, tag="bps")
                    if 1 <= kb <= 14:
                        nc.tensor.transpose(
                            b_ps[:, 0:128], bba_all[:, kb - 1, :], ident)
                        nc.tensor.transpose(
                            b_ps[:, 128:160], bbb_all[0:32, kb - 1, :],
                            ident[0:32, 0:32])
                    else:
                        bba = bst.tile([128, 128], fp16, tag="bba")
                        nc.sync.dma_start(
                            bba,
                            bass.AP(tensor=text_d[h].tensor, offset=base_off,
                                    ap=[[R - 1, 128], [1, 128]]))
                        nc.tensor.transpose(b_ps[:, 0:128], bba, ident)
                        n2 = nrows - 128
                        bbb = bst.tile([64, 128], fp16, tag="bbb")
                        nc.sync.dma_start(
                            bbb[0:n2, :],
                            bass.AP(tensor=text_d[h].tensor,
                                    offset=base_off + 128 * (R - 1),
                                    ap=[[R - 1, n2], [1, 128]]))
                        nc.tensor.transpose(
                            b_ps[:, 128:128 + n2], bbb[0:n2, :], ident[0:n2, 0:n2])
                    nc.scalar.activation(
                        bnd_all[:, h, kb, 0:nrows], b_ps[:, 0:nrows], AF.Copy)
            # whole mask in one chunked 3D DMA (emitted late so it doesn't
            # block the projection input loads on the DMA queues)
            nc.sync.dma_start(
                maskt_all,
                bass.AP(tensor=maskt_d.tensor, offset=0,
                        ap=[[S, 128], [128 * S, 16], [1, S]]))

        import os
        if os.environ.get("K_PHASE") == "setup":
            return
        # ---- main attention loop (head pairs row-packed on the PE) ----
        with tc.tile_pool(name="sp", bufs=2, space="PSUM") as sp, \
             tc.tile_pool(name="op", bufs=2, space="PSUM") as op, \
             tc.tile_pool(name="zp", bufs=2) as zp, \
             tc.tile_pool(name="mp", bufs=4) as mp, \
             tc.tile_pool(name="rp", bufs=3) as rp, \
             tc.tile_pool(name="tailp", bufs=2) as tailp:

            def emit_merge(eng, dst, h, kb, Q0, z_lo, z_hi):
                # dst [128,1024] fp16 = maskt + (zone | band) for cols [Q0,Q0+1024)
                j0 = kb * 128
                c1 = max(j0 - 16, 0)
                c2 = min(j0 + 144, S)
                Q1 = Q0 + 1024
                lo_a, lo_b = Q0, min(c1, Q1)
                if lo_a < lo_b:
                    eng.tensor_add(dst[:, lo_a - Q0:lo_b - Q0],
                                   maskt_all[:, kb, lo_a:lo_b], z_lo[:, lo_a:lo_b])
                bd_a, bd_b = max(c1, Q0), min(c2, Q1)
                if bd_a < bd_b:
                    eng.tensor_add(dst[:, bd_a - Q0:bd_b - Q0],
                                   maskt_all[:, kb, bd_a:bd_b],
                                   bnd_all[:, h, kb, bd_a - c1:bd_b - c1])
                hi_a, hi_b = max(c2, Q0), Q1
                if hi_a < hi_b:
                    eng.tensor_add(dst[:, hi_a - Q0:hi_b - Q0],
                                   maskt_all[:, kb, hi_a:hi_b], z_hi[:, hi_a:hi_b])

            for hp in range(2):
                # zone rows: strided column reads of Text, then broadcast.
                # left cols (i <= j-17) -> T[i,32] (Text col 160);
                # right cols (i >= j+17) -> T[i,0] (Text col 0)
                zs = []
                for e in range(2):
                    h = 2 * hp + e
                    z_lo = zp.tile([128, S], fp16, tag=f"zlo{e}", name=f"zlo{e}")
                    nc.sync.dma_start(z_lo[0:1, :], t01_d[h][1:2, :])
                    nc.gpsimd.partition_broadcast(z_lo, z_lo[0:1, :], channels=128)
                    z_hi = zp.tile([128, S], fp16, tag=f"zhi{e}", name=f"zhi{e}")
                    nc.sync.dma_start(z_hi[0:1, :], t01_d[h][0:1, :])
                    nc.gpsimd.partition_broadcast(z_hi, z_hi[0:1, :], channels=128)
                    zs.append((z_lo, z_hi))
                for qh in range(2):
                    Q0 = qh * 1024
                    Q1 = Q0 + 1024
                    oTs = [op.tile([AD + 1, 1024], f32, tag="ot", name=f"ot{e}")
                           for e in range(2)]
                    for kb in range(16):
                        j0 = kb * 128
                        s_tiles = []
                        for e in range(2):
                            h = 2 * hp + e
                            m_mg = mp.tile([128, 1024], fp16, tag="mg",
                                           name=f"mg{e}")
                            eng = nc.gpsimd if (kb + e) % 2 == 1 else nc.vector
                            emit_merge(eng, m_mg, h, kb, Q0, *zs[e])
                            s_ps = sp.tile([128, 1024], f32, tag="s", name=f"s{e}")
                            b = 64 * e
                            for q2 in range(2):
                                qs = Q0 + q2 * 512
                                nc.tensor.matmul(
                                    s_ps[:, q2 * 512:(q2 + 1) * 512],
                                    lhsT=kt[hp][b:b + 64, j0:j0 + 128],
                                    rhs=qt[hp][b:b + 64, qs:qs + 512],
                                    start=True, stop=False, skip_group_check=True)
                                nc.tensor.matmul(
                                    s_ps[:, q2 * 512:(q2 + 1) * 512],
                                    lhsT=ident,
                                    rhs=m_mg[:, q2 * 512:(q2 + 1) * 512],
                                    start=False, stop=True, skip_group_check=True)
                            s_tiles.append(s_ps)
                        for e in range(2):
                            h = 2 * hp + e
                            g0 = h * (AD + 1)
                            r_t = rp.tile([128, 1024], fp16, tag="r", name=f"r{e}")
                            nc.scalar.activation(r_t, s_tiles[e], AF.Relu,
                                                 bias=spb_bc[:, 0:1])
                            t_t = rp.tile([128, 1024], fp16, tag="t", name=f"t{e}")
                            nc.vector.tensor_mul(t_t, r_t, r_t)
                            for q2 in range(2):
                                nc.tensor.matmul(
                                    oTs[e][:, q2 * 512:(q2 + 1) * 512],
                                    lhsT=v_all[:, kb, g0:g0 + AD + 1],
                                    rhs=t_t[:, q2 * 512:(q2 + 1) * 512],
                                    start=(kb == 0), stop=(kb == 15),
                                    skip_group_check=True)
                    # normalization: rc = 1/(sum + eps), broadcast, scale
                    for e in range(2):
                        oT_ps = oTs[e]
                        rcb = tailp.tile([64, 1024], f32, tag="rcb", name="rcb")
                        nc.vector.tensor_scalar_add(
                            rcb[0:1, :], oT_ps[AD:AD + 1, :], EPS)
                        # unscaled fp16 copy releases the oT psum banks early
                        ou = tailp.tile([64, 1024], fp16, tag="ou", name="ou")
                        nc.scalar.activation(ou, oT_ps[0:AD, :], AF.Copy)
                        nc.vector.reciprocal(rcb[0:1, :], rcb[0:1, :])
                        nc.gpsimd.partition_broadcast(rcb, rcb[0:1, :], channels=64)
                        if e == 0:
                            nc.vector.tensor_mul(
                                oT_sb[hp][0:64, Q0:Q1], ou, rcb)
                        else:
                            ost = tailp.tile([64, 1024], fp16, tag="ost",
                                             name="ost")
                            nc.vector.tensor_mul(ost, ou, rcb)
                            nc.sync.dma_start(oT_sb[hp][64:128, Q0:Q1], ost)

        if os.environ.get("K_PHASE") == "nomain_out":
            pass
        # ---- output projection ----
        with tc.tile_pool(name="po", bufs=2, space="PSUM") as po, \
             tc.tile_pool(name="outp", bufs=3) as outp:
            for qb in range(16):
                o_ps = po.tile([128, DIN], f32, tag="o")
                for ci in range(2):
                    for n2 in range(2):
                        nc.tensor.matmul(
                            o_ps[:, n2 * 512:(n2 + 1) * 512],
                            lhsT=oT_sb[ci][:, qb * 128:(qb + 1) * 128],
                            rhs=wot_sb[ci][:, n2 * 512:(n2 + 1) * 512],
                            start=(ci == 0), stop=(ci == 1))
                outsb = outp.tile([128, DIN], f32, tag="outsb")
                nc.scalar.activation(outsb, o_ps, AF.Copy)
                nc.sync.dma_start(out_d[qb * 128:(qb + 1) * 128, :], outsb)


def _prep_core_inputs(c, iQ, iK, iV, mask, Wq, Wk, Wv, Wo, rel_emb, sp_bias):
    b, g = divmod(c, 4)
    hs = slice(g * NH * AD, (g + 1) * NH * AD)
    relt = np.ascontiguousarray(rel_emb.T).astype(np.float16)  # [64, 33]
    return {
        "iqt": np.ascontiguousarray(iQ[b].T).astype(np.float16),
        "ikt": np.ascontiguousarray(iK[b].T).astype(np.float16),
        "ivt": np.ascontiguousarray(iV[b].T).astype(np.float16),
        "maskt": np.where(mask[b].T, np.float16(MASKVAL), np.float16(0.0)),
        "wqt": np.ascontiguousarray((Wq[hs, :] / 8.0).T).astype(np.float16),
        "wkt": np.ascontiguousarray(Wk[hs, :].T).astype(np.float16),
        "wvt": np.ascontiguousarray(Wv[hs, :].T).astype(np.float16),
        "wot": np.ascontiguousarray(Wo[:, hs].T).astype(np.float16),
        "relt": np.concatenate([relt, relt], axis=0),  # [128, 33] (both halves)
        "ident": np.eye(128, dtype=np.float16),
        "spb": np.asarray(sp_bias, dtype=np.float32).reshape(1, 1),
    }


def kernel(iQ, iK, iV, mask, Wq, Wk, Wv, Wo, rel_emb, sp_bias, _trace=False,
           **_run_kwargs):
    iQ, iK, iV = (np.asarray(x, np.float32) for x in (iQ, iK, iV))
    mask = np.asarray(mask, bool)
    Wq, Wk, Wv, Wo = (np.asarray(x, np.float32) for x in (Wq, Wk, Wv, Wo))
    rel_emb = np.asarray(rel_emb, np.float32)
    sp_bias = np.asarray(sp_bias, np.float32)

    if "nc" not in _NC_CACHE:
        _NC_CACHE["nc"] = build_nc()
    nc = _NC_CACHE["nc"]

    in_maps = [
        _prep_core_inputs(c, iQ, iK, iV, mask, Wq, Wk, Wv, Wo, rel_emb, sp_bias)
        for c in range(8)
    ]
    res = run_bass_kernel_spmd(nc, in_maps, core_ids=list(range(8)),
                               trace=_trace, **_run_kwargs)
    out = np.zeros((2, S, DIN), np.float32)
    for c in range(8):
        bi = c // 4
        out[bi] += res.results[c]["out"]
    kernel.last_result = res
    return out


# revision 37
# speedup vs baseline: 14985.2534x; 1.0170x over previous
"""Trainium2 Bass kernel for nn_MultiHeadAttn (sparse_attention).

Self-contained: accepts FULL unsharded inputs, returns FULL output.

Sharding: batch (2) x head-groups (4 heads each) -> 8 cores.
Each core computes, for its (batch b, heads 4g..4g+3):
  q/k projections -> qT/kT [d=64 per head, S] fp16 (pair-stacked tiles)
  v projection    -> v [S, 64 per head] fp16 (+ ones column for row-sums)
  scores^T[j,i] = k_j . q_i  (PE, K=64, per 128-row key block)
  + rel-pos bias + additive mask, injected via identity-matmul accumulation
  relu(x+spb)^2 (ACT relu + DVE square), unnormalized AV on PE,
  normalization folded into a post-AV per-query scale,
  output projection -> partial [S, 1024] f32, summed across head-groups on host.

Rel-pos bias B^T[j,i] = T[i, clamp(j-i,-16,16)+16], T = (q/8) @ rel_emb^T:
  - middle band (|i-j|<=~16 + clamped staircase corners) is materialized
    per (head, key-block) via a diagonal-affine DMA read from a DRAM
    "Text" expansion of T, then PE-transposed.
  - constant zones (T[i,32] left / T[i,0] right) are row-broadcasts,
    materialized once per head via partition-step-0 broadcast DMA.
"""
import sys

sys.path.insert(0, "/opt/trn_rl_repo")

import numpy as np
import concourse.bass as bass
import concourse.tile as tile
from concourse import bacc, mybir
from concourse.bass_utils import run_bass_kernel_spmd

fp16 = mybir.dt.float16
f32 = mybir.dt.float32
AF = mybir.ActivationFunctionType

S = 2048          # sequence length
DIN = 1024        # model dim
NH = 4            # heads per core
AD = 64           # head dim
KREL = 16
R = 288           # Text row stride (287 used + 1 pad)
MASKVAL = -30000.0
EPS = 1e-9

_NC_CACHE = {}


def build_nc():
    nc = bacc.Bacc("TRN2", target_bir_lowering=False, debug=False, num_devices=8)

    iqt_d = nc.dram_tensor("iqt", [DIN, S], fp16, kind="ExternalInput").ap()
    ikt_d = nc.dram_tensor("ikt", [DIN, S], fp16, kind="ExternalInput").ap()
    ivt_d = nc.dram_tensor("ivt", [DIN, S], fp16, kind="ExternalInput").ap()
    maskt_d = nc.dram_tensor("maskt", [S, S], fp16, kind="ExternalInput").ap()
    wqt_d = nc.dram_tensor("wqt", [DIN, NH * AD], fp16, kind="ExternalInput").ap()
    wkt_d = nc.dram_tensor("wkt", [DIN, NH * AD], fp16, kind="ExternalInput").ap()
    wvt_d = nc.dram_tensor("wvt", [DIN, NH * AD], fp16, kind="ExternalInput").ap()
    wot_d = nc.dram_tensor("wot", [NH * AD, DIN], fp16, kind="ExternalInput").ap()
    relt_d = nc.dram_tensor("relt", [128, 33], fp16, kind="ExternalInput").ap()
    ident_d = nc.dram_tensor("ident", [128, 128], fp16, kind="ExternalInput").ap()
    spb_d = nc.dram_tensor("spb", [1, 1], f32, kind="ExternalInput").ap()
    out_d = nc.dram_tensor("out", [S, DIN], f32, kind="ExternalOutput").ap()

    with tile.TileContext(nc, pool_alloc_mode="queue") as tc:
        _body(nc, tc, iqt_d, ikt_d, ivt_d, maskt_d, wqt_d, wkt_d, wvt_d, wot_d,
              relt_d, ident_d, spb_d, out_d)
    nc.compile()
    return nc


def _body(nc, tc, iqt_d, ikt_d, ivt_d, maskt_d, wqt_d, wkt_d, wvt_d, wot_d,
          relt_d, ident_d, spb_d, out_d):
    from contextlib import ExitStack
    with ExitStack() as ctx:
        dram = ctx.enter_context(tc.tile_pool(name="dram", bufs=1, space="DRAM"))
        singles = ctx.enter_context(tc.tile_pool(name="singles", bufs=1))

        text_d = [dram.tile([S, R], fp16, name=f"text{h}") for h in range(NH)]
        t01_d = [dram.tile([2, S], fp16, name=f"t01d{h}") for h in range(NH)]

        # ---- persistent SBUF ----
        maskt_all = singles.tile([128, 16, S], fp16)
        qt = [singles.tile([128, S], fp16, name=f"qt{p}") for p in range(2)]
        kt = [singles.tile([128, S], fp16, name=f"kt{p}") for p in range(2)]
        v_all = singles.tile([128, 16, NH * (AD + 1)], fp16)
        bnd_all = singles.tile([128, NH, 16, 160], fp16)
        oT_sb = [singles.tile([128, S], fp16, name=f"oTsb{p}") for p in range(2)]
        wot_sb = [singles.tile([128, DIN], fp16, name=f"wotsb{p}") for p in range(2)]
        ident = singles.tile([128, 128], fp16)
        relt_sb = singles.tile([128, 33], fp16)
        spb_bc = singles.tile([128, 1], f32)
        ones_t = singles.tile([128, 128], fp16)

        nc.sync.dma_start(ident, ident_d)
        nc.sync.dma_start(relt_sb, relt_d)
        nc.sync.dma_start(spb_bc[0:1, :], spb_d)
        nc.gpsimd.partition_broadcast(spb_bc, spb_bc[0:1, :], channels=128)
        nc.vector.memset(ones_t, 1.0)
        for p in range(2):
            nc.sync.dma_start(wot_sb[p], wot_d[p * 128:(p + 1) * 128, :])
        # ones columns of v_all (for row sums via augmented AV matmul)
        for h in range(NH):
            nc.vector.memset(v_all[:, :, h * (AD + 1) + AD:h * (AD + 1) + AD + 1], 1.0)

        def chunked_w(w_d):
            return bass.AP(tensor=w_d.tensor, offset=0,
                           ap=[[NH * AD, 128], [128 * NH * AD, 8], [1, NH * AD]])

        # ---- q/k projections ----
        with tc.tile_pool(name="pp", bufs=2, space="PSUM") as pp, \
             tc.tile_pool(name="stage", bufs=2) as stage, \
             tc.tile_pool(name="xstage", bufs=2) as xst:
            for (x_d, w_d, dst) in ((iqt_d, wqt_d, qt), (ikt_d, wkt_d, kt)):
                wstage = stage.tile([128, 8, NH * AD], fp16, tag="w")
                nc.sync.dma_start(wstage, chunked_w(w_d))
                ps = [pp.tile([128, S], f32, tag="pqk", name=f"ps{p}") for p in range(2)]
                for g in range(2):  # chunk-groups of 4, one staged load each
                    xt = xst.tile([128, 4, S], fp16, tag="x")
                    nc.sync.dma_start(
                        xt,
                        bass.AP(tensor=x_d.tensor, offset=g * 4 * 128 * S,
                                ap=[[S, 128], [128 * S, 4], [1, S]]))
                    for c4 in range(4):
                        ch = g * 4 + c4
                        for hp in range(2):
                            for n2 in range(4):
                                nc.tensor.matmul(
                                    ps[hp][:, n2 * 512:(n2 + 1) * 512],
                                    lhsT=wstage[:, ch, hp * 128:(hp + 1) * 128],
                                    rhs=xt[:, c4, n2 * 512:(n2 + 1) * 512],
                                    start=(ch == 0), stop=(ch == 7))
                for hp in range(2):
                    nc.scalar.activation(dst[hp], ps[hp], AF.Copy)

        # ---- v projection + T/Text/band, all concurrently scheduled ----
        # psum: pv 2x[128,256](2 banks) + T 2x(1) + tps 2x(1) + bps 2x(1) = 8
        with tc.tile_pool(name="pv", bufs=2, space="PSUM") as pv, \
             tc.tile_pool(name="vw", bufs=1) as vw, \
             tc.tile_pool(name="pt", bufs=2, space="PSUM") as pt, \
             tc.tile_pool(name="textp", bufs=3) as textp, \
             tc.tile_pool(name="tcolp", bufs=2) as tcolp, \
             tc.tile_pool(name="pb", bufs=2, space="PSUM") as pb, \
             tc.tile_pool(name="bst", bufs=3) as bst:
            wvstage = vw.tile([128, 8, NH * AD], fp16)
            nc.sync.dma_start(wvstage, chunked_w(wvt_d))
            xv_all = vw.tile([128, 8, S], fp16)
            nc.sync.dma_start(
                xv_all,
                bass.AP(tensor=ivt_d.tensor, offset=0,
                        ap=[[S, 128], [128 * S, 8], [1, S]]))
            for kb in range(16):
                v_ps = pv.tile([128, NH * AD], f32, tag="pv")
                for ch in range(8):
                    nc.tensor.matmul(
                        v_ps, lhsT=xv_all[:, ch, kb * 128:(kb + 1) * 128],
                        rhs=wvstage[:, ch, :], start=(ch == 0), stop=(ch == 7))
                nc.scalar.activation(
                    v_all[:, kb, :].rearrange("p (h c) -> p h c", c=AD + 1)[:, :, 0:AD],
                    v_ps.rearrange("p (h c) -> p h c", c=AD), AF.Copy)

            for h in range(NH):
                b = 64 * (h % 2)
                # T = q @ rel_emb^T; Text row expansion -> DRAM (4 chunks/DMA)
                tcols = tcolp.tile([128, 2, 16], f32, tag="tcols")
                for i4 in range(4):
                    text_t = textp.tile([128, 4, R], fp16, tag="text")
                    for c4 in range(4):
                        ic = i4 * 4 + c4
                        T_ps = pt.tile([128, 33], f32, tag="T")
                        nc.tensor.matmul(
                            T_ps, lhsT=qt[h // 2][b:b + 64, ic * 128:(ic + 1) * 128],
                            rhs=relt_sb[b:b + 64, :], start=True, stop=True)
                        tt = text_t[:, c4, :]
                        nc.scalar.activation(tt[:, 127:160], T_ps, AF.Copy)
                        nc.vector.tensor_copy(tcols[:, 0, ic:ic + 1], T_ps[:, 0:1])
                        nc.vector.tensor_copy(tcols[:, 1, ic:ic + 1], T_ps[:, 32:33])
                        nc.vector.tensor_scalar_mul(
                            tt[:, 0:127], ones_t[:, 0:127], tcols[:, 0, ic:ic + 1])
                        nc.vector.tensor_scalar_mul(
                            tt[:, 160:287], ones_t[:, 0:127], tcols[:, 1, ic:ic + 1])
                    nc.sync.dma_start(
                        bass.AP(tensor=text_d[h].tensor, offset=i4 * 4 * 128 * R,
                                ap=[[R, 128], [128 * R, 4], [1, R]]),
                        text_t)
                # transpose tcols -> [32, 128]: rows 0-15 = T0 segments,
                # rows 16-31 = T32 segments; ship rows to DRAM as [2, S]
                tcols16 = tcolp.tile([128, 32], fp16, tag="tcols16")
                nc.vector.tensor_copy(tcols16, tcols.rearrange("p a b -> p (a b)"))
                tps = pt.tile([32, 128], fp16, tag="tps")
                nc.tensor.transpose(tps, tcols16, ident)
                tsb = tcolp.tile([32, 128], fp16, tag="tsb")
                nc.vector.tensor_copy(tsb, tps)
                nc.sync.dma_start(t01_d[h], tsb)
                # band tiles: diagonal DMA reads of Text (batched) + PE transpose
                bba_all = bst.tile([128, 14, 128], fp16, tag="bba_all")
                nc.sync.dma_start(
                    bba_all,
                    bass.AP(tensor=text_d[h].tensor,
                            offset=112 * (R - 1) + 128 + 143,
                            ap=[[R - 1, 128], [128 * (R - 1) + 128, 14], [1, 128]]))
                bbb_all = bst.tile([32, 14, 128], fp16, tag="bbb_all")
                nc.sync.dma_start(
                    bbb_all,
                    bass.AP(tensor=text_d[h].tensor,
                            offset=(112 + 128) * (R - 1) + 128 + 143,
                            ap=[[R - 1, 32], [128 * (R - 1) + 128, 14], [1, 128]]))
                for kb in range(16):
                    j0 = kb * 128
                    i_start = max(j0 - 16, 0)
                    nrows = min(j0 + 144, S) - i_start
                    base_off = i_start * (R - 1) + j0 + 143
                    b_ps = pb.tile([128, 160], fp16, tag="bps")
                    if 1 <= kb <= 14:
                        nc.tensor.transpose(
                            b_ps[:, 0:128], bba_all[:, kb - 1, :], ident)
                        nc.tensor.transpose(
                            b_ps[:, 128:160], bbb_all[0:32, kb - 1, :],
                            ident[0:32, 0:32])
                    else:
                        bba = bst.tile([128, 128], fp16, tag="bba")
                        nc.sync.dma_start(
                            bba,
                            bass.AP(tensor=text_d[h].tensor, offset=base_off,
                                    ap=[[R - 1, 128], [1, 128]]))
                        nc.tensor.transpose(b_ps[:, 0:128], bba, ident)
                        n2 = nrows - 128
                        bbb = bst.tile([64, 128], fp16, tag="bbb")
                        nc.sync.dma_start(
                            bbb[0:n2, :],
                            bass.AP(tensor=text_d[h].tensor,
                                    offset=base_off + 128 * (R - 1),
                                    ap=[[R - 1, n2], [1, 128]]))
                        nc.tensor.transpose(
                            b_ps[:, 128:128 + n2], bbb[0:n2, :], ident[0:n2, 0:n2])
                    nc.scalar.activation(
                        bnd_all[:, h, kb, 0:nrows], b_ps[:, 0:nrows], AF.Copy)
            # whole mask in one chunked 3D DMA (emitted late so it doesn't
            # block the projection input loads on the DMA queues)
            nc.sync.dma_start(
                maskt_all,
                bass.AP(tensor=maskt_d.tensor, offset=0,
                        ap=[[S, 128], [128 * S, 16], [1, S]]))

        import os
        if os.environ.get("K_PHASE") == "setup":
            return
        # ---- main attention loop (head pairs row-packed on the PE) ----
        with tc.tile_pool(name="sp", bufs=2, space="PSUM") as sp, \
             tc.tile_pool(name="op", bufs=2, space="PSUM") as op, \
             tc.tile_pool(name="zp", bufs=2) as zp, \
             tc.tile_pool(name="mp", bufs=6) as mp, \
             tc.tile_pool(name="rp", bufs=4) as rp, \
             tc.tile_pool(name="tailp", bufs=2) as tailp:

            def emit_merge(eng, dst, h, kb, Q0, z_lo, z_hi):
                # dst [128,1024] fp16 = maskt + (zone | band) for cols [Q0,Q0+1024)
                j0 = kb * 128
                c1 = max(j0 - 16, 0)
                c2 = min(j0 + 144, S)
                Q1 = Q0 + 1024
                lo_a, lo_b = Q0, min(c1, Q1)
                if lo_a < lo_b:
                    eng.tensor_add(dst[:, lo_a - Q0:lo_b - Q0],
                                   maskt_all[:, kb, lo_a:lo_b], z_lo[:, lo_a:lo_b])
                bd_a, bd_b = max(c1, Q0), min(c2, Q1)
                if bd_a < bd_b:
                    eng.tensor_add(dst[:, bd_a - Q0:bd_b - Q0],
                                   maskt_all[:, kb, bd_a:bd_b],
                                   bnd_all[:, h, kb, bd_a - c1:bd_b - c1])
                hi_a, hi_b = max(c2, Q0), Q1
                if hi_a < hi_b:
                    eng.tensor_add(dst[:, hi_a - Q0:hi_b - Q0],
                                   maskt_all[:, kb, hi_a:hi_b], z_hi[:, hi_a:hi_b])

            for hp in range(2):
                # zone rows: strided column reads of Text, then broadcast.
                # left cols (i <= j-17) -> T[i,32] (Text col 160);
                # right cols (i >= j+17) -> T[i,0] (Text col 0)
                zs = []
                for e in range(2):
                    h = 2 * hp + e
                    z_lo = zp.tile([128, S], fp16, tag=f"zlo{e}", name=f"zlo{e}")
                    nc.sync.dma_start(z_lo[0:1, :], t01_d[h][1:2, :])
                    nc.gpsimd.partition_broadcast(z_lo, z_lo[0:1, :], channels=128)
                    z_hi = zp.tile([128, S], fp16, tag=f"zhi{e}", name=f"zhi{e}")
                    nc.sync.dma_start(z_hi[0:1, :], t01_d[h][0:1, :])
                    nc.gpsimd.partition_broadcast(z_hi, z_hi[0:1, :], channels=128)
                    zs.append((z_lo, z_hi))
                for qh in range(2):
                    Q0 = qh * 1024
                    Q1 = Q0 + 1024
                    oTs = [op.tile([AD + 1, 1024], f32, tag="ot", name=f"ot{e}")
                           for e in range(2)]
                    for kb in range(16):
                        j0 = kb * 128
                        s_tiles = []
                        for e in range(2):
                            h = 2 * hp + e
                            m_mg = mp.tile([128, 1024], fp16, tag="mg",
                                           name=f"mg{e}")
                            eng = nc.gpsimd if (kb + e) % 2 == 1 else nc.vector
                            emit_merge(eng, m_mg, h, kb, Q0, *zs[e])
                            s_ps = sp.tile([128, 1024], f32, tag="s", name=f"s{e}")
                            b = 64 * e
                            for q2 in range(2):
                                qs = Q0 + q2 * 512
                                nc.tensor.matmul(
                                    s_ps[:, q2 * 512:(q2 + 1) * 512],
                                    lhsT=kt[hp][b:b + 64, j0:j0 + 128],
                                    rhs=qt[hp][b:b + 64, qs:qs + 512],
                                    start=True, stop=False, skip_group_check=True)
                                nc.tensor.matmul(
                                    s_ps[:, q2 * 512:(q2 + 1) * 512],
                                    lhsT=ident,
                                    rhs=m_mg[:, q2 * 512:(q2 + 1) * 512],
                                    start=False, stop=True, skip_group_check=True)
                            s_tiles.append(s_ps)
                        for e in range(2):
                            h = 2 * hp + e
                            g0 = h * (AD + 1)
                            r_t = rp.tile([128, 1024], fp16, tag="r", name=f"r{e}")
                            nc.scalar.activation(r_t, s_tiles[e], AF.Relu,
                                                 bias=spb_bc[:, 0:1])
                            t_t = rp.tile([128, 1024], fp16, tag="t", name=f"t{e}")
                            nc.vector.tensor_mul(t_t, r_t, r_t)
                            for q2 in range(2):
                                nc.tensor.matmul(
                                    oTs[e][:, q2 * 512:(q2 + 1) * 512],
                                    lhsT=v_all[:, kb, g0:g0 + AD + 1],
                                    rhs=t_t[:, q2 * 512:(q2 + 1) * 512],
                                    start=(kb == 0), stop=(kb == 15),
                                    skip_group_check=True)
                    # normalization: rc = 1/(sum + eps), broadcast, scale
                    for e in range(2):
                        oT_ps = oTs[e]
                        rcb = tailp.tile([64, 1024], f32, tag="rcb", name="rcb")
                        nc.vector.tensor_scalar_add(
                            rcb[0:1, :], oT_ps[AD:AD + 1, :], EPS)
                        # unscaled fp16 copy releases the oT psum banks early
                        ou = tailp.tile([64, 1024], fp16, tag="ou", name="ou")
                        nc.scalar.activation(ou, oT_ps[0:AD, :], AF.Copy)
                        nc.vector.reciprocal(rcb[0:1, :], rcb[0:1, :])
                        nc.gpsimd.partition_broadcast(rcb, rcb[0:1, :], channels=64)
                        if e == 0:
                            nc.vector.tensor_mul(
                                oT_sb[hp][0:64, Q0:Q1], ou, rcb)
                        else:
                            ost = tailp.tile([64, 1024], fp16, tag="ost",
                                             name="ost")
                            nc.vector.tensor_mul(ost, ou, rcb)
                            nc.sync.dma_start(oT_sb[hp][64:128, Q0:Q1], ost)

        if os.environ.get("K_PHASE") == "nomain_out":
            pass
        # ---- output projection ----
        with tc.tile_pool(name="po", bufs=2, space="PSUM") as po, \
             tc.tile_pool(name="outp", bufs=3) as outp:
            for qb in range(16):
                o_ps = po.tile([128, DIN], f32, tag="o")
                for ci in range(2):
                    for n2 in range(2):
                        nc.tensor.matmul(
                            o_ps[:, n2 * 512:(n2 + 1) * 512],
                            lhsT=oT_sb[ci][:, qb * 128:(qb + 1) * 128],
                            rhs=wot_sb[ci][:, n2 * 512:(n2 + 1) * 512],
                            start=(ci == 0), stop=(ci == 1))
                outsb = outp.tile([128, DIN], f32, tag="outsb")
                nc.scalar.activation(outsb, o_ps, AF.Copy)
                nc.sync.dma_start(out_d[qb * 128:(qb + 1) * 128, :], outsb)


def _prep_core_inputs(c, iQ, iK, iV, mask, Wq, Wk, Wv, Wo, rel_emb, sp_bias):
    b, g = divmod(c, 4)
    hs = slice(g * NH * AD, (g + 1) * NH * AD)
    relt = np.ascontiguousarray(rel_emb.T).astype(np.float16)  # [64, 33]
    return {
        "iqt": np.ascontiguousarray(iQ[b].T).astype(np.float16),
        "ikt": np.ascontiguousarray(iK[b].T).astype(np.float16),
        "ivt": np.ascontiguousarray(iV[b].T).astype(np.float16),
        "maskt": np.where(mask[b].T, np.float16(MASKVAL), np.float16(0.0)),
        "wqt": np.ascontiguousarray((Wq[hs, :] / 8.0).T).astype(np.float16),
        "wkt": np.ascontiguousarray(Wk[hs, :].T).astype(np.float16),
        "wvt": np.ascontiguousarray(Wv[hs, :].T).astype(np.float16),
        "wot": np.ascontiguousarray(Wo[:, hs].T).astype(np.float16),
        "relt": np.concatenate([relt, relt], axis=0),  # [128, 33] (both halves)
        "ident": np.eye(128, dtype=np.float16),
        "spb": np.asarray(sp_bias, dtype=np.float32).reshape(1, 1),
    }


def kernel(iQ, iK, iV, mask, Wq, Wk, Wv, Wo, rel_emb, sp_bias, _trace=False,
           **_run_kwargs):
    iQ, iK, iV = (np.asarray(x, np.float32) for x in (iQ, iK, iV))
    mask = np.asarray(mask, bool)
    Wq, Wk, Wv, Wo = (np.asarray(x, np.float32) for x in (Wq, Wk, Wv, Wo))
    rel_emb = np.asarray(rel_emb, np.float32)
    sp_bias = np.asarray(sp_bias, np.float32)

    if "nc" not in _NC_CACHE:
        _NC_CACHE["nc"] = build_nc()
    nc = _NC_CACHE["nc"]

    in_maps = [
        _prep_core_inputs(c, iQ, iK, iV, mask, Wq, Wk, Wv, Wo, rel_emb, sp_bias)
        for c in range(8)
    ]
    res = run_bass_kernel_spmd(nc, in_maps, core_ids=list(range(8)),
                               trace=_trace, **_run_kwargs)
    out = np.zeros((2, S, DIN), np.float32)
    for c in range(8):
        bi = c // 4
        out[bi] += res.results[c]["out"]
    kernel.last_result = res
    return out
